# revision 5
# baseline (speedup 1.0000x reference)
"""Trainium2 Bass kernel for nn_Class1ProcessingModel (ragged peptide model).

Self-contained: takes FULL inputs (as produced by setup_inputs), shards the
batch over 8 NeuronCores, runs a Bass/Tile kernel via run_bass_kernel_spmd,
and gathers the full [B] output.

Math restructuring vs the reference (all exact up to dtype):
  - conv1d(SAME, K=9, CIN=21, F=128) over L=35 == matmuls contracting over
    contiguous 189-row windows of the per-row flattened [L*CIN] feature vec.
    Windows are split at 128-row chunk boundaries; leading misalignment is
    handled with zero-padded weight rows (PE operands must start at
    partition 0).
  - tanh is monotonic => masked max-pools commute with tanh; only 1 tanh per
    scalar output needed.  (y+1)*mask max trick == plain masked max since the
    windows are never empty (plen >= 5).
  - c-flank average window always has exactly 10 valid positions, so
    sum/cnt-1 with the +1 offset reduces to a plain mean of relu(conv).
  - the flank-average dense layers (F->1) commute with the position mean, so
    both flank averages are 1-column matmul projections; the n-side mean is
    PSUM-accumulated across its 10 fixed positions.
  - the 128->2 branch head and F->1 projections run with the *data* as the
    stationary operand, so per-row scalars land as [128 rows, col] in PSUM:
    batch ends up on partitions with no transposes and no extra drains.
"""

import os
import numpy as np
import ml_dtypes

B, L, CIN = 32768, 35, 21
N_FLANK, C_FLANK, PEP_MAX = 10, 10, 15
F, K, H = 128, 9, 64
N_CORES = 8
BSH = B // N_CORES          # rows per core
NB = 512                    # batch-tile (matmul moving free dim)
NT = BSH // NB              # batch tiles per core
NG = NB // 128              # 128-row groups per batch tile
NGT = BSH // 128            # groups per core (= NT*NG)
FLAT = L * CIN              # 735
NCH = 6                     # feature chunks of 128
FPAD = NCH * 128            # 768, padded flat features
BIG = 1.0e9

_CACHE = {}


def _piece_table():
    """Conv matmul pieces. For output position l the contraction range is
    [21*(l-4), 21*(l+5)) clipped to [0, 735). Each intersecting 128-chunk
    becomes one piece starting at chunk partition 0: (chunk, kdim, lead,
    wbase) with `lead` zero-padded weight rows at the top."""
    pieces = []
    for l in range(L):
        s = CIN * (l - K // 2)
        lo, hi = max(0, s), min(FLAT, s + K * CIN)
        pl = []
        for c in range(NCH):
            r0, r1 = 128 * c, min(hi, 128 * c + 128)
            if r1 > max(lo, r0):
                lead = max(0, lo - r0)
                pl.append((c, r1 - r0, lead, r0 + lead - s))
        pieces.append(pl)
    return pieces


def _build_program():
    import concourse.bass as bass
    import concourse.mybir as mybir
    import concourse.tile as tile

    dt = mybir.dt
    AF = mybir.ActivationFunctionType
    OP = mybir.AluOpType
    pieces = _piece_table()
    NP = sum(len(p) for p in pieces)

    nc = bass.Bass()
    seq_d = nc.declare_dram_parameter("seq", [BSH, FPAD], dt.bfloat16, isOutput=False)
    plen_d = nc.declare_dram_parameter("plen", [128, NGT], dt.float32, isOutput=False)
    wpack_d = nc.declare_dram_parameter("wpack", [128, NP * 128], dt.bfloat16, isOutput=False)
    w1_d = nc.declare_dram_parameter("w1", [128, 128], dt.bfloat16, isOutput=False)
    w2_d = nc.declare_dram_parameter("w2", [128, 2], dt.bfloat16, isOutput=False)
    vcat_d = nc.declare_dram_parameter("vcat", [128, 2], dt.bfloat16, isOutput=False)
    convb_d = nc.declare_dram_parameter("convb", [128, 1], dt.float32, isOutput=False)
    b1_d = nc.declare_dram_parameter("b1", [128, 1], dt.float32, isOutput=False)
    iona_d = nc.declare_dram_parameter("iona", [128, NG * 15], dt.float32, isOutput=False)
    ioc_d = nc.declare_dram_parameter("ioc", [128, NG * 15], dt.float32, isOutput=False)
    iow1_d = nc.declare_dram_parameter("iow1", [128, NG * 20], dt.float32, isOutput=False)
    iow2_d = nc.declare_dram_parameter("iow2", [128, NG * 20], dt.float32, isOutput=False)
    b6_d = nc.declare_dram_parameter("b6", [128, NG * 6], dt.float32, isOutput=False)
    w6_d = nc.declare_dram_parameter("w6", [128, NG * 6], dt.float32, isOutput=False)
    out_d = nc.declare_dram_parameter("out", [128, NGT], dt.float32, isOutput=True)
    out_bias = float(_CACHE["out_b"])

    with tile.TileContext(nc) as tc:
        with (
            tc.tile_pool(name="persist", bufs=1) as pp,
            tc.tile_pool(name="cr", bufs=2) as crp,
            tc.tile_pool(name="hid", bufs=2) as hidp,
            tc.tile_pool(name="wk", bufs=2) as wkp,
            tc.tile_pool(name="cvps", bufs=3, space="PSUM") as cvps,
            tc.tile_pool(name="hps", bufs=2, space="PSUM") as hps,
            tc.tile_pool(name="stps", bufs=2, space="PSUM") as stps,
        ):
            # ---- persistent tiles + const loads ----
            xt = [pp.tile([128, BSH], dt.bfloat16, tag=f"xt{c}", name=f"xt{c}") for c in range(NCH)]
            wpack = pp.tile([128, NP * 128], dt.bfloat16, tag="wpack")
            w1 = pp.tile([128, 128], dt.bfloat16, tag="w1")
            w2 = pp.tile([128, 2], dt.bfloat16, tag="w2")
            vcat = pp.tile([128, 2], dt.bfloat16, tag="vcat")
            convb = pp.tile([128, 1], dt.float32, tag="convb")
            b1 = pp.tile([128, 1], dt.float32, tag="b1")
            iona = pp.tile([128, NG * 15], dt.float32, tag="iona")
            ioc = pp.tile([128, NG * 15], dt.float32, tag="ioc")
            iow1 = pp.tile([128, NG * 20], dt.float32, tag="iow1")
            iow2 = pp.tile([128, NG * 20], dt.float32, tag="iow2")
            b6 = pp.tile([128, NG * 6], dt.float32, tag="b6")
            w6 = pp.tile([128, NG * 6], dt.float32, tag="w6")
            plen = pp.tile([128, NGT], dt.float32, tag="plen")
            osb = pp.tile([128, NGT], dt.float32, tag="osb")

            for tdst, tsrc in [(wpack, wpack_d), (w1, w1_d), (w2, w2_d),
                               (vcat, vcat_d), (convb, convb_d), (b1, b1_d),
                               (iona, iona_d), (ioc, ioc_d),
                               (iow1, iow1_d), (iow2, iow2_d), (b6, b6_d),
                               (w6, w6_d), (plen, plen_d)]:
                nc.sync.dma_start(tdst[:], tsrc[:])

            for t in range(NT):
                bc = slice(t * NB, (t + 1) * NB)
                # ---- load+transpose this batch tile's features ----
                for c in range(NCH):
                    nc.sync.dma_start(
                        xt[c][:, bc], seq_d[bc, c * 128:(c + 1) * 128],
                        transpose=True)

                # ---- conv ----
                cr = crp.tile([128, L * NB], dt.bfloat16, tag="cr")
                pj = 0
                for l in range(L):
                    cv = cvps.tile([128, NB], dt.float32, tag="cv")
                    pl = pieces[l]
                    for j, (c, kd, lead, wb) in enumerate(pl):
                        nc.tensor.matmul(
                            cv[:, :],
                            lhsT=wpack[0:kd, pj * 128:pj * 128 + 128],
                            rhs=xt[c][0:kd, bc],
                            start=(j == 0), stop=(j == len(pl) - 1))
                        pj += 1
                    dst = cr[:, l * NB:(l + 1) * NB]
                    if l % 2 == 0:
                        nc.scalar.activation(dst, cv[:, :], AF.Relu, bias=convb[:, 0:1])
                    else:
                        nc.vector.tensor_scalar(dst, cv[:, :], convb[:, 0:1], 0.0,
                                                OP.add, OP.max)

                # ---- branch layer 1 (both branches fused) ----
                hid = hidp.tile([128, 15 * NB], dt.bfloat16, tag="hid")
                for i, l in enumerate(range(10, 25)):
                    hp = hps.tile([128, NB], dt.float32, tag="hp")
                    nc.tensor.matmul(hp[:, :], lhsT=w1[:, :],
                                     rhs=cr[:, l * NB:(l + 1) * NB],
                                     start=True, stop=True)
                    dst = hid[:, i * NB:(i + 1) * NB]
                    if i % 2 == 0:
                        nc.vector.tensor_scalar(dst, hp[:, :], b1[:, 0:1], 0.0,
                                                OP.add, OP.max)
                    else:
                        nc.scalar.activation(dst, hp[:, :], AF.Relu, bias=b1[:, 0:1])

                # ---- per-row scalars: branch head + projections ----
                # st[:, g*51 + c]: cols 0..29 = z_n/z_c interleaved (l=10+i),
                # 30..49 = cavg projection (l=15+j), 50 = n-flank avg proj.
                st = stps.tile([128, NG * 51], dt.float32, tag="st")
                for g in range(NG):
                    gb = slice(g * 128, (g + 1) * 128)
                    for i in range(15):
                        nc.tensor.matmul(
                            st[:, g * 51 + 2 * i:g * 51 + 2 * i + 2],
                            lhsT=hid[:, i * NB + g * 128:i * NB + (g + 1) * 128],
                            rhs=w2[:, 0:2], start=True, stop=True)
                    for j, l in enumerate(range(15, 35)):
                        nc.tensor.matmul(
                            st[:, g * 51 + 30 + j:g * 51 + 31 + j],
                            lhsT=cr[:, l * NB + g * 128:l * NB + (g + 1) * 128],
                            rhs=vcat[:, 1:2], start=True, stop=True)
                    for j, l in enumerate(range(0, 10)):
                        nc.tensor.matmul(
                            st[:, g * 51 + 50:g * 51 + 51],
                            lhsT=cr[:, l * NB + g * 128:l * NB + (g + 1) * 128],
                            rhs=vcat[:, 0:1], start=(j == 0), stop=(j == 9))

                # ---- ragged finale (batch on partitions) ----
                prep = wkp.tile([128, NG * 20], dt.float32, tag="prep")
                for g in range(NG):
                    G = t * NG + g
                    nc.scalar.activation(prep[:, g * 20:(g + 1) * 20],
                                         iow1[:, 0:20], AF.Identity,
                                         bias=plen[:, G:G + 1], scale=0.0)

                stv = st[:].rearrange("p (g c) -> p g c", c=51)
                zn = stv[:, :, 0:30:2]
                zc = stv[:, :, 1:31:2]
                qc = stv[:, :, 30:50]
                prv = prep[:].rearrange("p (g c) -> p g c", c=20)
                pr15 = prv[:, :, 0:15]

                def w3(tag, w=15):
                    return wkp.tile([128, NG * w], dt.float32, tag=tag, name=tag)[:].rearrange(
                        "p (g c) -> p g c", c=w)

                t6 = wkp.tile([128, NG * 6], dt.float32, tag="t6")
                t6v = t6[:].rearrange("p (g c) -> p g c", c=6)
                ionav = iona[:].rearrange("p (g c) -> p g c", c=15)
                iocv = ioc[:].rearrange("p (g c) -> p g c", c=15)
                iow1v = iow1[:].rearrange("p (g c) -> p g c", c=20)
                iow2v = iow2[:].rearrange("p (g c) -> p g c", c=20)

                # n-branch masked max  -> t6 col 1
                mn = w3("mn")
                nc.vector.tensor_tensor(mn, ionav, pr15, OP.is_lt)
                tz = w3("tz")
                nc.vector.tensor_tensor(tz, zn, mn, OP.mult)
                pen = w3("pen")
                nc.vector.tensor_scalar(pen, mn, BIG, BIG, OP.mult, OP.subtract)
                nc.vector.tensor_tensor(tz, tz, pen, OP.add)
                nc.vector.reduce_max(t6v[:, :, 1:2], tz, axis=mybir.AxisListType.X)

                # c-branch masked max -> col 4 ; cleaved_c one-hot -> col 3
                mc = w3("mc")
                nc.vector.tensor_tensor(mc, iocv, pr15, OP.is_lt)
                oh = w3("oh")
                nc.vector.tensor_tensor(oh, iocv, pr15, OP.is_equal)
                tz2 = w3("tz2")
                nc.vector.tensor_tensor(tz2, zc, mc, OP.mult)
                pen2 = w3("pen2")
                nc.vector.tensor_scalar(pen2, mc, BIG, BIG, OP.mult, OP.subtract)
                nc.vector.tensor_tensor(tz2, tz2, pen2, OP.add)
                nc.vector.reduce_max(t6v[:, :, 4:5], tz2, axis=mybir.AxisListType.X)
                sel = w3("sel")
                nc.vector.tensor_tensor(sel, zc, oh, OP.mult)
                nc.vector.reduce_sum(t6v[:, :, 3:4], sel, axis=mybir.AxisListType.X)

                # c-flank window mean (pre-projected) -> col 5
                mw1 = w3("mw1", 20)
                nc.vector.tensor_tensor(mw1, iow1v, prv, OP.is_ge)
                mw2 = w3("mw2", 20)
                nc.vector.tensor_tensor(mw2, iow2v, prv, OP.is_lt)
                nc.vector.tensor_tensor(mw1, mw1, mw2, OP.mult)
                qt = w3("qt", 20)
                nc.vector.tensor_tensor(qt, qc, mw1, OP.mult)
                nc.vector.reduce_sum(t6v[:, :, 5:6], qt, axis=mybir.AxisListType.X)

                # cleaved_n -> col 0 ; n-flank avg projection -> col 2
                nc.scalar.copy(t6v[:, :, 0:1], stv[:, :, 0:1])
                nc.scalar.copy(t6v[:, :, 2:3], stv[:, :, 50:51])

                # biases, tanh, output weights, sigmoid
                nc.vector.tensor_tensor(t6[:, :], t6[:, :], b6[:, :], OP.add)
                t6b = wkp.tile([128, NG * 6], dt.float32, tag="t6b")
                nc.scalar.activation(t6b[:, :], t6[:, :], AF.Tanh)
                nc.vector.tensor_tensor(t6b[:, :], t6b[:, :], w6[:, :], OP.mult)
                s1 = wkp.tile([128, NG], dt.float32, tag="s1")
                nc.vector.reduce_sum(s1[:, :], t6b[:].rearrange("p (g c) -> p g c", c=6),
                                     axis=mybir.AxisListType.X)
                nc.scalar.activation(osb[:, t * NG:(t + 1) * NG], s1[:, :],
                                     AF.Sigmoid, bias=out_bias)

            nc.sync.dma_start(out_d[:], osb[:])

    _split_excess_waits(nc)
    return nc


def _split_excess_waits(nc, max_waits=1):
    """This walrus build rejects instructions carrying multiple sync waits
    (the TileContext tail drain gets the whole global clock attached).
    Move excess waits onto injected same-engine NoOps just before."""
    import concourse.mybir as mybir
    for f in nc.m.functions:
        for bb in f.blocks:
            out, changed = [], False
            for inst in bb.instructions:
                si = inst.sync_info
                waits = list(si.on_wait) if si and si.on_wait else []
                if len(waits) > max_waits:
                    extra, keep = waits[:-max_waits], waits[-max_waits:]
                    for i in range(0, len(extra), max_waits):
                        nop = mybir.InstNoOp(name=f"{inst.name}-wsplit-{i}",
                                             ins=[], outs=[])
                        nop.engine = inst.engine
                        nop.sync_info = mybir.SyncInfo(
                            on_wait=extra[i:i + max_waits], on_update=[])
                        out.append(nop)
                    inst.sync_info = mybir.SyncInfo(
                        on_wait=keep,
                        on_update=list(si.on_update) if si.on_update else [])
                    changed = True
                out.append(inst)
            if changed:
                bb.instructions = out


def _host_consts(conv_w, conv_b, n_w1, n_b1, n_w2, n_b2, c_w1, c_b1, c_w2,
                 c_b2, navg_w, navg_b, cavg_w, cavg_b, out_w, out_b):
    bf16 = ml_dtypes.bfloat16
    pieces = _piece_table()
    NP = sum(len(p) for p in pieces)
    wflat = np.asarray(conv_w, np.float32).reshape(K * CIN, F)
    wpack = np.zeros((128, NP * 128), np.float32)
    pj = 0
    for l in range(L):
        for (c, kd, lead, wb) in pieces[l]:
            wpack[lead:kd, pj * 128:pj * 128 + 128] = wflat[wb:wb + kd - lead, :]
            pj += 1
    w1 = np.concatenate([np.asarray(n_w1, np.float32),
                         np.asarray(c_w1, np.float32)], axis=1)      # [128,128]
    w2 = np.zeros((128, 2), np.float32)
    w2[0:H, 0] = np.asarray(n_w2, np.float32)[:, 0]
    w2[H:128, 1] = np.asarray(c_w2, np.float32)[:, 0]
    vcat = np.stack([np.asarray(navg_w, np.float32)[:, 0] / N_FLANK,
                     np.asarray(cavg_w, np.float32)[:, 0] / C_FLANK], axis=1)
    b1cat = np.concatenate([np.asarray(n_b1, np.float32),
                            np.asarray(c_b1, np.float32)])[:, None]
    rep = lambda row: np.tile(np.asarray(row, np.float32)[None, :], (128, NG)).copy()
    j15 = np.arange(15, dtype=np.float32)
    iona = j15.copy(); iona[0] = BIG
    ioc = j15 + 1.0
    j20 = np.arange(20, dtype=np.float32)
    ow = np.asarray(out_w, np.float32)[:, 0] * np.array([1, -1, 1, 1, -1, 1], np.float32)
    sc = lambda x: float(np.asarray(x).reshape(-1)[0])
    b6 = np.array([sc(n_b2), sc(n_b2), sc(navg_b),
                   sc(c_b2), sc(c_b2), sc(cavg_b)], np.float32)
    return {
        "wpack": wpack.astype(bf16),
        "w1": w1.astype(bf16),
        "w2": w2.astype(bf16),
        "vcat": vcat.astype(bf16),
        "convb": np.asarray(conv_b, np.float32)[:, None].copy(),
        "b1": b1cat.copy(),
        "iona": rep(iona),
        "ioc": rep(ioc),
        "iow1": rep(j20 + 5.0),
        "iow2": rep(j20 - 5.0),
        "b6": rep(b6),
        "w6": rep(ow),
    }, sc(out_b)


def kernel(**inputs):
    from concourse.bass_utils import run_bass_kernel_spmd

    seq = np.asarray(inputs["sequence"], np.float32)
    plen = np.asarray(inputs["peptide_length"], np.int32)
    consts, out_bias = _host_consts(
        inputs["conv_w"], inputs["conv_b"], inputs["n_w1"], inputs["n_b1"],
        inputs["n_w2"], inputs["n_b2"], inputs["c_w1"], inputs["c_b1"],
        inputs["c_w2"], inputs["c_b2"], inputs["navg_w"], inputs["navg_b"],
        inputs["cavg_w"], inputs["cavg_b"], inputs["out_w"], inputs["out_b"])

    if "nc" not in _CACHE:
        _CACHE["out_b"] = out_bias
        _CACHE["nc"] = _build_program()
    nc = _CACHE["nc"]

    seq_flat = np.zeros((B, FPAD), np.float32)
    seq_flat[:, :FLAT] = seq.reshape(B, FLAT)
    seq_bf = seq_flat.astype(ml_dtypes.bfloat16)
    plen_f = plen.astype(np.float32)

    in_maps = []
    for i in range(N_CORES):
        sh = slice(i * BSH, (i + 1) * BSH)
        m = dict(consts)
        m["seq"] = np.ascontiguousarray(seq_bf[sh])
        m["plen"] = np.ascontiguousarray(plen_f[sh].reshape(NGT, 128).T)
        in_maps.append(m)

    trace = bool(int(os.environ.get("TRN_KERNEL_TRACE", "0")))
    res = run_bass_kernel_spmd(nc, in_maps, list(range(N_CORES)), trace=trace)
    if trace and res.exec_time_ns is not None:
        print(f"HW exec time: {res.exec_time_ns} ns")
        _CACHE["exec_time_ns"] = res.exec_time_ns
        _CACHE["profile"] = res

    out = np.empty((B,), np.float32)
    for i in range(N_CORES):
        arr = np.asarray(res.results[i]["out"], np.float32)   # [128, NGT]
        out[i * BSH:(i + 1) * BSH] = arr.T.reshape(-1)
    return out


# revision 7
# speedup vs baseline: 130.1277x; 130.1277x over previous
"""Trainium2 Bass kernel for nn_Class1ProcessingModel (ragged peptide model).

Self-contained: takes FULL inputs (as produced by setup_inputs), shards the
batch over 8 NeuronCores, runs a Bass/Tile kernel via run_bass_kernel_spmd,
and gathers the full [B] output.

Math restructuring vs the reference (all exact up to dtype):
  - conv1d(SAME, K=9, CIN=21, F=128) over L=35 == matmuls contracting over
    contiguous 189-row windows of the per-row flattened [L*CIN] feature vec.
    Windows are split at 128-row chunk boundaries; leading misalignment is
    handled with zero-padded weight rows (PE operands must start at
    partition 0).
  - tanh is monotonic => masked max-pools commute with tanh; only 1 tanh per
    scalar output needed.  (y+1)*mask max trick == plain masked max since the
    windows are never empty (plen >= 5).
  - c-flank average window always has exactly 10 valid positions, so
    sum/cnt-1 with the +1 offset reduces to a plain mean of relu(conv).
  - the flank-average dense layers (F->1) commute with the position mean, so
    both flank averages are 1-column matmul projections; the n-side mean is
    PSUM-accumulated across its 10 fixed positions.
  - the 128->2 branch head and F->1 projections run with the *data* as the
    stationary operand, so per-row scalars land as [128 rows, col] in PSUM:
    batch ends up on partitions with no transposes and no extra drains.
"""

import os
import numpy as np
import ml_dtypes

B, L, CIN = 32768, 35, 21
N_FLANK, C_FLANK, PEP_MAX = 10, 10, 15
F, K, H = 128, 9, 64
N_CORES = 8
BSH = B // N_CORES          # rows per core
NB = 512                    # batch-tile (matmul moving free dim)
NT = BSH // NB              # batch tiles per core
NG = NB // 128              # 128-row groups per batch tile
NGT = BSH // 128            # groups per core (= NT*NG)
FLAT = L * CIN              # 735
NCH = 6                     # feature chunks of 128
FPAD = NCH * 128            # 768, padded flat features
BIG = 1.0e9

_CACHE = {}


def _piece_table():
    """Conv matmul pieces. For output position l the contraction range is
    [21*(l-4), 21*(l+5)) clipped to [0, 735). Each intersecting 128-chunk
    becomes one piece starting at chunk partition 0: (chunk, kdim, lead,
    wbase) with `lead` zero-padded weight rows at the top."""
    pieces = []
    for l in range(L):
        s = CIN * (l - K // 2)
        lo, hi = max(0, s), min(FLAT, s + K * CIN)
        pl = []
        for c in range(NCH):
            r0, r1 = 128 * c, min(hi, 128 * c + 128)
            if r1 > max(lo, r0):
                lead = max(0, lo - r0)
                pl.append((c, r1 - r0, lead, r0 + lead - s))
        pieces.append(pl)
    return pieces


def _build_program(repeat=1):
    import contextlib
    import concourse.bass as bass
    import concourse.mybir as mybir
    import concourse.tile as tile

    dt = mybir.dt
    AF = mybir.ActivationFunctionType
    OP = mybir.AluOpType
    pieces = _piece_table()
    NP = sum(len(p) for p in pieces)

    nc = bass.Bass()
    seq_d = nc.declare_dram_parameter("seq", [BSH, FPAD], dt.bfloat16, isOutput=False)
    plen_d = nc.declare_dram_parameter("plen", [128, NGT], dt.float32, isOutput=False)
    wpack_d = nc.declare_dram_parameter("wpack", [128, NP * 128], dt.bfloat16, isOutput=False)
    w1_d = nc.declare_dram_parameter("w1", [128, 128], dt.bfloat16, isOutput=False)
    w2_d = nc.declare_dram_parameter("w2", [128, 2], dt.bfloat16, isOutput=False)
    vcat_d = nc.declare_dram_parameter("vcat", [128, 2], dt.bfloat16, isOutput=False)
    convb_d = nc.declare_dram_parameter("convb", [128, 1], dt.float32, isOutput=False)
    b1_d = nc.declare_dram_parameter("b1", [128, 1], dt.float32, isOutput=False)
    iona_d = nc.declare_dram_parameter("iona", [128, NG * 15], dt.float32, isOutput=False)
    ioc_d = nc.declare_dram_parameter("ioc", [128, NG * 15], dt.float32, isOutput=False)
    iow1_d = nc.declare_dram_parameter("iow1", [128, NG * 20], dt.float32, isOutput=False)
    iow2_d = nc.declare_dram_parameter("iow2", [128, NG * 20], dt.float32, isOutput=False)
    b6_d = nc.declare_dram_parameter("b6", [128, NG * 6], dt.float32, isOutput=False)
    w6_d = nc.declare_dram_parameter("w6", [128, NG * 6], dt.float32, isOutput=False)
    out_d = nc.declare_dram_parameter("out", [128, NGT], dt.float32, isOutput=True)
    out_bias = float(_CACHE["out_b"])

    with tile.TileContext(nc) as tc:
        with (
            tc.tile_pool(name="persist", bufs=1) as pp,
            tc.tile_pool(name="cr", bufs=2) as crp,
            tc.tile_pool(name="hid", bufs=2) as hidp,
            tc.tile_pool(name="wk", bufs=2) as wkp,
            tc.tile_pool(name="cvps", bufs=3, space="PSUM") as cvps,
            tc.tile_pool(name="hps", bufs=2, space="PSUM") as hps,
            tc.tile_pool(name="stps", bufs=2, space="PSUM") as stps,
        ):
            # ---- persistent tiles + const loads ----
            xt = [pp.tile([128, BSH], dt.bfloat16, tag=f"xt{c}", name=f"xt{c}") for c in range(NCH)]
            wpack = pp.tile([128, NP * 128], dt.bfloat16, tag="wpack")
            w1 = pp.tile([128, 128], dt.bfloat16, tag="w1")
            w2 = pp.tile([128, 2], dt.bfloat16, tag="w2")
            vcat = pp.tile([128, 2], dt.bfloat16, tag="vcat")
            convb = pp.tile([128, 1], dt.float32, tag="convb")
            b1 = pp.tile([128, 1], dt.float32, tag="b1")
            iona = pp.tile([128, NG * 15], dt.float32, tag="iona")
            ioc = pp.tile([128, NG * 15], dt.float32, tag="ioc")
            iow1 = pp.tile([128, NG * 20], dt.float32, tag="iow1")
            iow2 = pp.tile([128, NG * 20], dt.float32, tag="iow2")
            b6 = pp.tile([128, NG * 6], dt.float32, tag="b6")
            w6 = pp.tile([128, NG * 6], dt.float32, tag="w6")
            plen = pp.tile([128, NGT], dt.float32, tag="plen")
            osb = pp.tile([128, NGT], dt.float32, tag="osb")

            for tdst, tsrc in [(wpack, wpack_d), (w1, w1_d), (w2, w2_d),
                               (vcat, vcat_d), (convb, convb_d), (b1, b1_d),
                               (iona, iona_d), (ioc, ioc_d),
                               (iow1, iow1_d), (iow2, iow2_d), (b6, b6_d),
                               (w6, w6_d), (plen, plen_d)]:
                nc.sync.dma_start(tdst[:], tsrc[:])

            rep_ctx = tc.For_i(0, repeat, 1) if repeat > 1 else contextlib.nullcontext()
            with rep_ctx:
              for t in range(NT):
                bc = slice(t * NB, (t + 1) * NB)
                # ---- load+transpose this batch tile's features ----
                for c in range(NCH):
                    nc.sync.dma_start(
                        xt[c][:, bc], seq_d[bc, c * 128:(c + 1) * 128],
                        transpose=True)

                # ---- conv ----
                cr = crp.tile([128, L * NB], dt.bfloat16, tag="cr")
                pj = 0
                for l in range(L):
                    cv = cvps.tile([128, NB], dt.float32, tag="cv")
                    pl = pieces[l]
                    for j, (c, kd, lead, wb) in enumerate(pl):
                        nc.tensor.matmul(
                            cv[:, :],
                            lhsT=wpack[0:kd, pj * 128:pj * 128 + 128],
                            rhs=xt[c][0:kd, bc],
                            start=(j == 0), stop=(j == len(pl) - 1))
                        pj += 1
                    dst = cr[:, l * NB:(l + 1) * NB]
                    if l % 2 == 0:
                        nc.scalar.activation(dst, cv[:, :], AF.Relu, bias=convb[:, 0:1])
                    else:
                        nc.vector.tensor_scalar(dst, cv[:, :], convb[:, 0:1], 0.0,
                                                OP.add, OP.max)

                # ---- branch layer 1 (both branches fused) ----
                hid = hidp.tile([128, 15 * NB], dt.bfloat16, tag="hid")
                for i, l in enumerate(range(10, 25)):
                    hp = hps.tile([128, NB], dt.float32, tag="hp")
                    nc.tensor.matmul(hp[:, :], lhsT=w1[:, :],
                                     rhs=cr[:, l * NB:(l + 1) * NB],
                                     start=True, stop=True)
                    dst = hid[:, i * NB:(i + 1) * NB]
                    if i % 2 == 0:
                        nc.vector.tensor_scalar(dst, hp[:, :], b1[:, 0:1], 0.0,
                                                OP.add, OP.max)
                    else:
                        nc.scalar.activation(dst, hp[:, :], AF.Relu, bias=b1[:, 0:1])

                # ---- per-row scalars: branch head + projections ----
                # st[:, g*51 + c]: cols 0..29 = z_n/z_c interleaved (l=10+i),
                # 30..49 = cavg projection (l=15+j), 50 = n-flank avg proj.
                st = stps.tile([128, NG * 51], dt.float32, tag="st")
                for g in range(NG):
                    gb = slice(g * 128, (g + 1) * 128)
                    for i in range(15):
                        nc.tensor.matmul(
                            st[:, g * 51 + 2 * i:g * 51 + 2 * i + 2],
                            lhsT=hid[:, i * NB + g * 128:i * NB + (g + 1) * 128],
                            rhs=w2[:, 0:2], start=True, stop=True)
                    for j, l in enumerate(range(15, 35)):
                        nc.tensor.matmul(
                            st[:, g * 51 + 30 + j:g * 51 + 31 + j],
                            lhsT=cr[:, l * NB + g * 128:l * NB + (g + 1) * 128],
                            rhs=vcat[:, 1:2], start=True, stop=True)
                    for j, l in enumerate(range(0, 10)):
                        nc.tensor.matmul(
                            st[:, g * 51 + 50:g * 51 + 51],
                            lhsT=cr[:, l * NB + g * 128:l * NB + (g + 1) * 128],
                            rhs=vcat[:, 0:1], start=(j == 0), stop=(j == 9))

                # ---- ragged finale (batch on partitions) ----
                prep = wkp.tile([128, NG * 20], dt.float32, tag="prep")
                for g in range(NG):
                    G = t * NG + g
                    nc.scalar.activation(prep[:, g * 20:(g + 1) * 20],
                                         iow1[:, 0:20], AF.Identity,
                                         bias=plen[:, G:G + 1], scale=0.0)

                stv = st[:].rearrange("p (g c) -> p g c", c=51)
                zn = stv[:, :, 0:30:2]
                zc = stv[:, :, 1:31:2]
                qc = stv[:, :, 30:50]
                prv = prep[:].rearrange("p (g c) -> p g c", c=20)
                pr15 = prv[:, :, 0:15]

                def w3(tag, w=15):
                    return wkp.tile([128, NG * w], dt.float32, tag=tag, name=tag)[:].rearrange(
                        "p (g c) -> p g c", c=w)

                t6 = wkp.tile([128, NG * 6], dt.float32, tag="t6")
                t6v = t6[:].rearrange("p (g c) -> p g c", c=6)
                ionav = iona[:].rearrange("p (g c) -> p g c", c=15)
                iocv = ioc[:].rearrange("p (g c) -> p g c", c=15)
                iow1v = iow1[:].rearrange("p (g c) -> p g c", c=20)
                iow2v = iow2[:].rearrange("p (g c) -> p g c", c=20)

                # n-branch masked max  -> t6 col 1
                mn = w3("mn")
                nc.vector.tensor_tensor(mn, ionav, pr15, OP.is_lt)
                tz = w3("tz")
                nc.vector.tensor_tensor(tz, zn, mn, OP.mult)
                pen = w3("pen")
                nc.vector.tensor_scalar(pen, mn, BIG, BIG, OP.mult, OP.subtract)
                nc.vector.tensor_tensor(tz, tz, pen, OP.add)
                nc.vector.reduce_max(t6v[:, :, 1:2], tz, axis=mybir.AxisListType.X)

                # c-branch masked max -> col 4 ; cleaved_c one-hot -> col 3
                mc = w3("mc")
                nc.vector.tensor_tensor(mc, iocv, pr15, OP.is_lt)
                oh = w3("oh")
                nc.vector.tensor_tensor(oh, iocv, pr15, OP.is_equal)
                tz2 = w3("tz2")
                nc.vector.tensor_tensor(tz2, zc, mc, OP.mult)
                pen2 = w3("pen2")
                nc.vector.tensor_scalar(pen2, mc, BIG, BIG, OP.mult, OP.subtract)
                nc.vector.tensor_tensor(tz2, tz2, pen2, OP.add)
                nc.vector.reduce_max(t6v[:, :, 4:5], tz2, axis=mybir.AxisListType.X)
                sel = w3("sel")
                nc.vector.tensor_tensor(sel, zc, oh, OP.mult)
                nc.vector.reduce_sum(t6v[:, :, 3:4], sel, axis=mybir.AxisListType.X)

                # c-flank window mean (pre-projected) -> col 5
                mw1 = w3("mw1", 20)
                nc.vector.tensor_tensor(mw1, iow1v, prv, OP.is_ge)
                mw2 = w3("mw2", 20)
                nc.vector.tensor_tensor(mw2, iow2v, prv, OP.is_lt)
                nc.vector.tensor_tensor(mw1, mw1, mw2, OP.mult)
                qt = w3("qt", 20)
                nc.vector.tensor_tensor(qt, qc, mw1, OP.mult)
                nc.vector.reduce_sum(t6v[:, :, 5:6], qt, axis=mybir.AxisListType.X)

                # cleaved_n -> col 0 ; n-flank avg projection -> col 2
                nc.scalar.copy(t6v[:, :, 0:1], stv[:, :, 0:1])
                nc.scalar.copy(t6v[:, :, 2:3], stv[:, :, 50:51])

                # biases, tanh, output weights, sigmoid
                nc.vector.tensor_tensor(t6[:, :], t6[:, :], b6[:, :], OP.add)
                t6b = wkp.tile([128, NG * 6], dt.float32, tag="t6b")
                nc.scalar.activation(t6b[:, :], t6[:, :], AF.Tanh)
                nc.vector.tensor_tensor(t6b[:, :], t6b[:, :], w6[:, :], OP.mult)
                s1 = wkp.tile([128, NG], dt.float32, tag="s1")
                nc.vector.reduce_sum(s1[:, :], t6b[:].rearrange("p (g c) -> p g c", c=6),
                                     axis=mybir.AxisListType.X)
                nc.scalar.activation(osb[:, t * NG:(t + 1) * NG], s1[:, :],
                                     AF.Sigmoid, bias=out_bias)

            nc.sync.dma_start(out_d[:], osb[:])

    _split_excess_waits(nc)
    return nc


def _split_excess_waits(nc, max_waits=1):
    """This walrus build rejects instructions carrying multiple sync waits
    (the TileContext tail drain gets the whole global clock attached).
    Move excess waits onto injected same-engine NoOps just before."""
    import concourse.mybir as mybir
    for f in nc.m.functions:
        for bb in f.blocks:
            out, changed = [], False
            for inst in bb.instructions:
                si = inst.sync_info
                waits = list(si.on_wait) if si and si.on_wait else []
                if len(waits) > max_waits:
                    extra, keep = waits[:-max_waits], waits[-max_waits:]
                    for i in range(0, len(extra), max_waits):
                        nop = mybir.InstNoOp(name=f"{inst.name}-wsplit-{i}",
                                             ins=[], outs=[])
                        nop.engine = inst.engine
                        nop.sync_info = mybir.SyncInfo(
                            on_wait=extra[i:i + max_waits], on_update=[])
                        out.append(nop)
                    inst.sync_info = mybir.SyncInfo(
                        on_wait=keep,
                        on_update=list(si.on_update) if si.on_update else [])
                    changed = True
                out.append(inst)
            if changed:
                bb.instructions = out


def _host_consts(conv_w, conv_b, n_w1, n_b1, n_w2, n_b2, c_w1, c_b1, c_w2,
                 c_b2, navg_w, navg_b, cavg_w, cavg_b, out_w, out_b):
    bf16 = ml_dtypes.bfloat16
    pieces = _piece_table()
    NP = sum(len(p) for p in pieces)
    wflat = np.asarray(conv_w, np.float32).reshape(K * CIN, F)
    wpack = np.zeros((128, NP * 128), np.float32)
    pj = 0
    for l in range(L):
        for (c, kd, lead, wb) in pieces[l]:
            wpack[lead:kd, pj * 128:pj * 128 + 128] = wflat[wb:wb + kd - lead, :]
            pj += 1
    w1 = np.concatenate([np.asarray(n_w1, np.float32),
                         np.asarray(c_w1, np.float32)], axis=1)      # [128,128]
    w2 = np.zeros((128, 2), np.float32)
    w2[0:H, 0] = np.asarray(n_w2, np.float32)[:, 0]
    w2[H:128, 1] = np.asarray(c_w2, np.float32)[:, 0]
    vcat = np.stack([np.asarray(navg_w, np.float32)[:, 0] / N_FLANK,
                     np.asarray(cavg_w, np.float32)[:, 0] / C_FLANK], axis=1)
    b1cat = np.concatenate([np.asarray(n_b1, np.float32),
                            np.asarray(c_b1, np.float32)])[:, None]
    rep = lambda row: np.tile(np.asarray(row, np.float32)[None, :], (128, NG)).copy()
    j15 = np.arange(15, dtype=np.float32)
    iona = j15.copy(); iona[0] = BIG
    ioc = j15 + 1.0
    j20 = np.arange(20, dtype=np.float32)
    ow = np.asarray(out_w, np.float32)[:, 0] * np.array([1, -1, 1, 1, -1, 1], np.float32)
    sc = lambda x: float(np.asarray(x).reshape(-1)[0])
    b6 = np.array([sc(n_b2), sc(n_b2), sc(navg_b),
                   sc(c_b2), sc(c_b2), sc(cavg_b)], np.float32)
    return {
        "wpack": wpack.astype(bf16),
        "w1": w1.astype(bf16),
        "w2": w2.astype(bf16),
        "vcat": vcat.astype(bf16),
        "convb": np.asarray(conv_b, np.float32)[:, None].copy(),
        "b1": b1cat.copy(),
        "iona": rep(iona),
        "ioc": rep(ioc),
        "iow1": rep(j20 + 5.0),
        "iow2": rep(j20 - 5.0),
        "b6": rep(b6),
        "w6": rep(ow),
    }, sc(out_b)


def kernel(**inputs):
    from concourse.bass_utils import run_bass_kernel_spmd

    seq = np.asarray(inputs["sequence"], np.float32)
    plen = np.asarray(inputs["peptide_length"], np.int32)
    consts, out_bias = _host_consts(
        inputs["conv_w"], inputs["conv_b"], inputs["n_w1"], inputs["n_b1"],
        inputs["n_w2"], inputs["n_b2"], inputs["c_w1"], inputs["c_b1"],
        inputs["c_w2"], inputs["c_b2"], inputs["navg_w"], inputs["navg_b"],
        inputs["cavg_w"], inputs["cavg_b"], inputs["out_w"], inputs["out_b"])

    if "nc" not in _CACHE:
        _CACHE["out_b"] = out_bias
        _CACHE["nc"] = _build_program()
    nc = _CACHE["nc"]

    seq_flat = np.zeros((B, FPAD), np.float32)
    seq_flat[:, :FLAT] = seq.reshape(B, FLAT)
    seq_bf = seq_flat.astype(ml_dtypes.bfloat16)
    plen_f = plen.astype(np.float32)

    in_maps = []
    for i in range(N_CORES):
        sh = slice(i * BSH, (i + 1) * BSH)
        m = dict(consts)
        m["seq"] = np.ascontiguousarray(seq_bf[sh])
        m["plen"] = np.ascontiguousarray(plen_f[sh].reshape(NGT, 128).T)
        in_maps.append(m)

    trace = bool(int(os.environ.get("TRN_KERNEL_TRACE", "0")))
    res = run_bass_kernel_spmd(nc, in_maps, list(range(N_CORES)), trace=trace)
    if trace and res.exec_time_ns is not None:
        print(f"HW exec time: {res.exec_time_ns} ns")
        _CACHE["exec_time_ns"] = res.exec_time_ns
        _CACHE["profile"] = res

    out = np.empty((B,), np.float32)
    for i in range(N_CORES):
        arr = np.asarray(res.results[i]["out"], np.float32)   # [128, NGT]
        out[i * BSH:(i + 1) * BSH] = arr.T.reshape(-1)
    return out


# revision 19
# speedup vs baseline: 306.7326x; 2.3572x over previous
"""Trainium2 Bass kernel for nn_Class1ProcessingModel (ragged peptide model).

Self-contained: takes FULL inputs (as produced by setup_inputs), shards the
batch over 8 NeuronCores, runs a Bass/Tile kernel via run_bass_kernel_spmd,
and gathers the full [B] output.

Math restructuring vs the reference (all exact up to dtype):
  - conv1d(SAME, K=9, CIN=21, F=128) over L=35 == matmuls contracting over
    contiguous 189-row windows of the per-row flattened [L*CIN] feature vec.
    Windows are split at 128-row chunk boundaries; leading misalignment is
    handled with zero-padded weight rows (PE operands must start at
    partition 0).
  - tanh is monotonic => masked max-pools commute with tanh; only 1 tanh per
    scalar output needed.  (y+1)*mask max trick == plain masked max since the
    windows are never empty (plen >= 5).
  - c-flank average window always has exactly 10 valid positions, so
    sum/cnt-1 with the +1 offset reduces to a plain mean of relu(conv).
  - the flank-average dense layers (F->1) commute with the position mean, so
    both flank averages are 1-column matmul projections; the n-side mean is
    PSUM-accumulated across its 10 fixed positions.
  - the 128->2 branch head and F->1 projections run with the *data* as the
    stationary operand, so per-row scalars land as [128 rows, col] in PSUM:
    batch ends up on partitions with no transposes and no extra drains.
"""

import os
import numpy as np
import ml_dtypes

B, L, CIN = 32768, 35, 21
N_FLANK, C_FLANK, PEP_MAX = 10, 10, 15
F, K, H = 128, 9, 64
N_CORES = 8
BSH = B // N_CORES          # rows per core
NB = 512                    # batch-tile (matmul moving free dim)
NT = BSH // NB              # batch tiles per core
NG = NB // 128              # 128-row groups per batch tile
NGT = BSH // 128            # groups per core (= NT*NG)
FLAT = L * CIN              # 735
NCH = 6                     # feature chunks of 128
FPAD = NCH * 128            # 768, padded flat features
BIG = 1.0e9

_CACHE = {}


def _piece_table():
    """Conv matmul pieces. For output position l the contraction range is
    [21*(l-4), 21*(l+5)) clipped to [0, 735). Each intersecting 128-chunk
    becomes one piece starting at chunk partition 0: (chunk, kdim, lead,
    wbase) with `lead` zero-padded weight rows at the top.
    (A 21-aligned overlapping-tile variant with 66 instead of 80 matmuls
    was measured slower: it needs 35 DMA-transposes per batch tile and
    per-DMA dispatch overhead dominates.)"""
    pieces = []
    for l in range(L):
        s = CIN * (l - K // 2)
        lo, hi = max(0, s), min(FLAT, s + K * CIN)
        pl = []
        for c in range(NCH):
            r0, r1 = 128 * c, min(hi, 128 * c + 128)
            if r1 > max(lo, r0):
                lead = max(0, lo - r0)
                pl.append((c, r1 - r0, lead, r0 + lead - s))
        pieces.append(pl)
    return pieces


def _build_program(repeat=1):
    import contextlib
    import concourse.bass as bass
    import concourse.mybir as mybir
    import concourse.tile as tile

    dt = mybir.dt
    AF = mybir.ActivationFunctionType
    OP = mybir.AluOpType
    pieces = _piece_table()
    NP = sum(len(p) for p in pieces)

    nc = bass.Bass()
    seq_d = nc.declare_dram_parameter("seq", [BSH, FPAD], dt.bfloat16, isOutput=False)
    plen_d = nc.declare_dram_parameter("plen", [128, NGT], dt.float32, isOutput=False)
    wpack_d = nc.declare_dram_parameter("wpack", [128, NP * 128], dt.bfloat16, isOutput=False)
    w1_d = nc.declare_dram_parameter("w1", [128, 128], dt.bfloat16, isOutput=False)
    w2_d = nc.declare_dram_parameter("w2", [128, 2], dt.bfloat16, isOutput=False)
    vcat_d = nc.declare_dram_parameter("vcat", [128, 2], dt.bfloat16, isOutput=False)
    convb_d = nc.declare_dram_parameter("convb", [128, 1], dt.float32, isOutput=False)
    b1_d = nc.declare_dram_parameter("b1", [128, 1], dt.float32, isOutput=False)
    iona_d = nc.declare_dram_parameter("iona", [128, NG * 15], dt.float32, isOutput=False)
    ioc_d = nc.declare_dram_parameter("ioc", [128, NG * 15], dt.float32, isOutput=False)
    iow1_d = nc.declare_dram_parameter("iow1", [128, NG * 20], dt.float32, isOutput=False)
    iow2_d = nc.declare_dram_parameter("iow2", [128, NG * 20], dt.float32, isOutput=False)
    b6_d = nc.declare_dram_parameter("b6", [128, NG * 6], dt.float32, isOutput=False)
    w6_d = nc.declare_dram_parameter("w6", [128, NG * 6], dt.float32, isOutput=False)
    out_d = nc.declare_dram_parameter("out", [128, NGT], dt.float32, isOutput=True)
    out_bias = float(_CACHE["out_b"])

    with tile.TileContext(nc) as tc:
        with (
            tc.tile_pool(name="persist", bufs=1) as pp,
            tc.tile_pool(name="cr", bufs=2) as crp,
            tc.tile_pool(name="hid", bufs=2) as hidp,
            tc.tile_pool(name="wk", bufs=2) as wkp,
            tc.tile_pool(name="cvps", bufs=2, space="PSUM") as cvps,
            tc.tile_pool(name="hps", bufs=2, space="PSUM") as hps,
            tc.tile_pool(name="stps", bufs=2, space="PSUM") as stps,
        ):
            # ---- persistent tiles + const loads ----
            xt = [pp.tile([128, BSH], dt.bfloat16, tag=f"xt{c}", name=f"xt{c}")
                  for c in range(NCH)]
            wpack = pp.tile([128, NP * 128], dt.bfloat16, tag="wpack")
            w1 = pp.tile([128, 128], dt.bfloat16, tag="w1")
            w2 = pp.tile([128, 2], dt.bfloat16, tag="w2")
            vcat = pp.tile([128, 2], dt.bfloat16, tag="vcat")
            convb = pp.tile([128, 1], dt.float32, tag="convb")
            b1 = pp.tile([128, 1], dt.float32, tag="b1")
            iona = pp.tile([128, NG * 15], dt.float32, tag="iona")
            ioc = pp.tile([128, NG * 15], dt.float32, tag="ioc")
            iow1 = pp.tile([128, NG * 20], dt.float32, tag="iow1")
            iow2 = pp.tile([128, NG * 20], dt.float32, tag="iow2")
            b6 = pp.tile([128, NG * 6], dt.float32, tag="b6")
            w6 = pp.tile([128, NG * 6], dt.float32, tag="w6")
            plen = pp.tile([128, NGT], dt.float32, tag="plen")
            osb = pp.tile([128, NGT], dt.float32, tag="osb")

            for tdst, tsrc in [(wpack, wpack_d), (w1, w1_d), (w2, w2_d),
                               (vcat, vcat_d), (convb, convb_d), (b1, b1_d),
                               (iona, iona_d), (ioc, ioc_d),
                               (iow1, iow1_d), (iow2, iow2_d), (b6, b6_d),
                               (w6, w6_d), (plen, plen_d)]:
                nc.sync.dma_start(tdst[:], tsrc[:])

            rep_ctx = tc.For_i(0, repeat, 1) if repeat > 1 else contextlib.nullcontext()
            with rep_ctx:
              for t in range(NT):
                bc = slice(t * NB, (t + 1) * NB)
                # ---- load+transpose this batch tile's features ----
                for c in range(NCH):
                    nc.sync.dma_start(
                        xt[c][:, bc], seq_d[bc, c * 128:(c + 1) * 128],
                        transpose=True)

                # ---- conv (two positions share one 2-bank psum tile so the
                # relu+bias drains run as half as many, twice-as-wide ops) ----
                cr = crp.tile([128, L * NB], dt.bfloat16, tag="cr")
                pj = 0
                cv = None
                for l in range(L):
                    half = l % 2
                    if half == 0:
                        cv = cvps.tile([128, 2 * NB], dt.float32, tag="cv")
                    pl = pieces[l]
                    for j, (c, kd, lead, wb) in enumerate(pl):
                        nc.tensor.matmul(
                            cv[:, half * NB:(half + 1) * NB],
                            lhsT=wpack[0:kd, pj * 128:pj * 128 + 128],
                            rhs=xt[c][0:kd, bc],
                            start=(j == 0), stop=(j == len(pl) - 1))
                        pj += 1
                    if half == 1 or l == L - 1:
                        w = (half + 1) * NB
                        dst = cr[:, (l - half) * NB:(l - half) * NB + w]
                        if (l // 2) % 2 == 0:
                            nc.scalar.activation(dst, cv[:, 0:w], AF.Relu,
                                                 bias=convb[:, 0:1])
                        else:
                            nc.vector.tensor_scalar(dst, cv[:, 0:w],
                                                    convb[:, 0:1], 0.0,
                                                    OP.add, OP.max)

                # ---- branch layer 1 (both branches fused) ----
                hid = hidp.tile([128, 15 * NB], dt.bfloat16, tag="hid")
                for i, l in enumerate(range(10, 25)):
                    hp = hps.tile([128, NB], dt.float32, tag="hp")
                    nc.tensor.matmul(hp[:, :], lhsT=w1[:, :],
                                     rhs=cr[:, l * NB:(l + 1) * NB],
                                     start=True, stop=True)
                    dst = hid[:, i * NB:(i + 1) * NB]
                    if i % 2 == 0:
                        nc.vector.tensor_scalar(dst, hp[:, :], b1[:, 0:1], 0.0,
                                                OP.add, OP.max)
                    else:
                        nc.scalar.activation(dst, hp[:, :], AF.Relu, bias=b1[:, 0:1])

                # ---- per-row scalars: branch head + projections ----
                # st[:, g*51 + c]: cols 0..29 = z_n/z_c interleaved (l=10+i),
                # 30..49 = cavg projection (l=15+j), 50 = n-flank avg proj.
                st = stps.tile([128, NG * 51], dt.float32, tag="st")
                for g in range(NG):
                    gb = slice(g * 128, (g + 1) * 128)
                    for i in range(15):
                        nc.tensor.matmul(
                            st[:, g * 51 + 2 * i:g * 51 + 2 * i + 2],
                            lhsT=hid[:, i * NB + g * 128:i * NB + (g + 1) * 128],
                            rhs=w2[:, 0:2], start=True, stop=True)
                    for j, l in enumerate(range(15, 35)):
                        nc.tensor.matmul(
                            st[:, g * 51 + 30 + j:g * 51 + 31 + j],
                            lhsT=cr[:, l * NB + g * 128:l * NB + (g + 1) * 128],
                            rhs=vcat[:, 1:2], start=True, stop=True)
                    for j, l in enumerate(range(0, 10)):
                        nc.tensor.matmul(
                            st[:, g * 51 + 50:g * 51 + 51],
                            lhsT=cr[:, l * NB + g * 128:l * NB + (g + 1) * 128],
                            rhs=vcat[:, 0:1], start=(j == 0), stop=(j == 9))

                # ---- ragged finale (batch on partitions) ----
                prep = wkp.tile([128, NG * 20], dt.float32, tag="prep")
                for g in range(NG):
                    G = t * NG + g
                    nc.scalar.activation(prep[:, g * 20:(g + 1) * 20],
                                         iow1[:, 0:20], AF.Identity,
                                         bias=plen[:, G:G + 1], scale=0.0)

                stv = st[:].rearrange("p (g c) -> p g c", c=51)
                zn = stv[:, :, 0:30:2]
                zc = stv[:, :, 1:31:2]
                qc = stv[:, :, 30:50]
                prv = prep[:].rearrange("p (g c) -> p g c", c=20)
                pr15 = prv[:, :, 0:15]

                def w3(tag, w=15):
                    return wkp.tile([128, NG * w], dt.float32, tag=tag, name=tag)[:].rearrange(
                        "p (g c) -> p g c", c=w)

                t6 = wkp.tile([128, NG * 6], dt.float32, tag="t6")
                t6v = t6[:].rearrange("p (g c) -> p g c", c=6)
                ionav = iona[:].rearrange("p (g c) -> p g c", c=15)
                iocv = ioc[:].rearrange("p (g c) -> p g c", c=15)
                iow1v = iow1[:].rearrange("p (g c) -> p g c", c=20)
                iow2v = iow2[:].rearrange("p (g c) -> p g c", c=20)

                # n-branch masked max  -> t6 col 1
                mn = w3("mn")
                nc.vector.tensor_tensor(mn, ionav, pr15, OP.is_lt)
                tz = w3("tz")
                nc.vector.tensor_tensor(tz, zn, mn, OP.mult)
                pen = w3("pen")
                nc.vector.tensor_scalar(pen, mn, BIG, BIG, OP.mult, OP.subtract)
                nc.vector.tensor_tensor(tz, tz, pen, OP.add)
                nc.vector.reduce_max(t6v[:, :, 1:2], tz, axis=mybir.AxisListType.X)

                # c-branch masked max -> col 4 ; cleaved_c one-hot -> col 3
                mc = w3("mc")
                nc.vector.tensor_tensor(mc, iocv, pr15, OP.is_lt)
                oh = w3("oh")
                nc.vector.tensor_tensor(oh, iocv, pr15, OP.is_equal)
                tz2 = w3("tz2")
                nc.vector.tensor_tensor(tz2, zc, mc, OP.mult)
                pen2 = w3("pen2")
                nc.vector.tensor_scalar(pen2, mc, BIG, BIG, OP.mult, OP.subtract)
                nc.vector.tensor_tensor(tz2, tz2, pen2, OP.add)
                nc.vector.reduce_max(t6v[:, :, 4:5], tz2, axis=mybir.AxisListType.X)
                sel = w3("sel")
                nc.vector.tensor_tensor(sel, zc, oh, OP.mult)
                nc.vector.reduce_sum(t6v[:, :, 3:4], sel, axis=mybir.AxisListType.X)

                # c-flank window mean (pre-projected) -> col 5
                mw1 = w3("mw1", 20)
                nc.vector.tensor_tensor(mw1, iow1v, prv, OP.is_ge)
                mw2 = w3("mw2", 20)
                nc.vector.tensor_tensor(mw2, iow2v, prv, OP.is_lt)
                nc.vector.tensor_tensor(mw1, mw1, mw2, OP.mult)
                qt = w3("qt", 20)
                nc.vector.tensor_tensor(qt, qc, mw1, OP.mult)
                nc.vector.reduce_sum(t6v[:, :, 5:6], qt, axis=mybir.AxisListType.X)

                # cleaved_n -> col 0 ; n-flank avg projection -> col 2
                nc.scalar.copy(t6v[:, :, 0:1], stv[:, :, 0:1])
                nc.scalar.copy(t6v[:, :, 2:3], stv[:, :, 50:51])

                # biases, tanh, output weights, sigmoid
                nc.vector.tensor_tensor(t6[:, :], t6[:, :], b6[:, :], OP.add)
                t6b = wkp.tile([128, NG * 6], dt.float32, tag="t6b")
                nc.scalar.activation(t6b[:, :], t6[:, :], AF.Tanh)
                nc.vector.tensor_tensor(t6b[:, :], t6b[:, :], w6[:, :], OP.mult)
                s1 = wkp.tile([128, NG], dt.float32, tag="s1")
                nc.vector.reduce_sum(s1[:, :], t6b[:].rearrange("p (g c) -> p g c", c=6),
                                     axis=mybir.AxisListType.X)
                nc.scalar.activation(osb[:, t * NG:(t + 1) * NG], s1[:, :],
                                     AF.Sigmoid, bias=out_bias)

            nc.sync.dma_start(out_d[:], osb[:])

    _split_excess_waits(nc)
    return nc


def _split_excess_waits(nc, max_waits=1):
    """This walrus build rejects instructions carrying multiple sync waits
    (the TileContext tail drain gets the whole global clock attached).
    Move excess waits onto injected same-engine NoOps just before."""
    import concourse.mybir as mybir
    for f in nc.m.functions:
        for bb in f.blocks:
            out, changed = [], False
            for inst in bb.instructions:
                si = inst.sync_info
                waits = list(si.on_wait) if si and si.on_wait else []
                if len(waits) > max_waits:
                    extra, keep = waits[:-max_waits], waits[-max_waits:]
                    for i in range(0, len(extra), max_waits):
                        nop = mybir.InstNoOp(name=f"{inst.name}-wsplit-{i}",
                                             ins=[], outs=[])
                        nop.engine = inst.engine
                        nop.sync_info = mybir.SyncInfo(
                            on_wait=extra[i:i + max_waits], on_update=[])
                        out.append(nop)
                    inst.sync_info = mybir.SyncInfo(
                        on_wait=keep,
                        on_update=list(si.on_update) if si.on_update else [])
                    changed = True
                out.append(inst)
            if changed:
                bb.instructions = out


def _host_consts(conv_w, conv_b, n_w1, n_b1, n_w2, n_b2, c_w1, c_b1, c_w2,
                 c_b2, navg_w, navg_b, cavg_w, cavg_b, out_w, out_b):
    bf16 = ml_dtypes.bfloat16
    pieces = _piece_table()
    NP = sum(len(p) for p in pieces)
    wflat = np.asarray(conv_w, np.float32).reshape(K * CIN, F)
    wpack = np.zeros((128, NP * 128), np.float32)
    pj = 0
    for l in range(L):
        for (c, kd, lead, wb) in pieces[l]:
            wpack[lead:kd, pj * 128:pj * 128 + 128] = wflat[wb:wb + kd - lead, :]
            pj += 1
    w1 = np.concatenate([np.asarray(n_w1, np.float32),
                         np.asarray(c_w1, np.float32)], axis=1)      # [128,128]
    w2 = np.zeros((128, 2), np.float32)
    w2[0:H, 0] = np.asarray(n_w2, np.float32)[:, 0]
    w2[H:128, 1] = np.asarray(c_w2, np.float32)[:, 0]
    vcat = np.stack([np.asarray(navg_w, np.float32)[:, 0] / N_FLANK,
                     np.asarray(cavg_w, np.float32)[:, 0] / C_FLANK], axis=1)
    b1cat = np.concatenate([np.asarray(n_b1, np.float32),
                            np.asarray(c_b1, np.float32)])[:, None]
    rep = lambda row: np.tile(np.asarray(row, np.float32)[None, :], (128, NG)).copy()
    j15 = np.arange(15, dtype=np.float32)
    iona = j15.copy(); iona[0] = BIG
    ioc = j15 + 1.0
    j20 = np.arange(20, dtype=np.float32)
    ow = np.asarray(out_w, np.float32)[:, 0] * np.array([1, -1, 1, 1, -1, 1], np.float32)
    sc = lambda x: float(np.asarray(x).reshape(-1)[0])
    b6 = np.array([sc(n_b2), sc(n_b2), sc(navg_b),
                   sc(c_b2), sc(c_b2), sc(cavg_b)], np.float32)
    return {
        "wpack": wpack.astype(bf16),
        "w1": w1.astype(bf16),
        "w2": w2.astype(bf16),
        "vcat": vcat.astype(bf16),
        "convb": np.asarray(conv_b, np.float32)[:, None].copy(),
        "b1": b1cat.copy(),
        "iona": rep(iona),
        "ioc": rep(ioc),
        "iow1": rep(j20 + 5.0),
        "iow2": rep(j20 - 5.0),
        "b6": rep(b6),
        "w6": rep(ow),
    }, sc(out_b)


def kernel(**inputs):
    from concourse.bass_utils import run_bass_kernel_spmd

    seq = np.asarray(inputs["sequence"], np.float32)
    plen = np.asarray(inputs["peptide_length"], np.int32)
    consts, out_bias = _host_consts(
        inputs["conv_w"], inputs["conv_b"], inputs["n_w1"], inputs["n_b1"],
        inputs["n_w2"], inputs["n_b2"], inputs["c_w1"], inputs["c_b1"],
        inputs["c_w2"], inputs["c_b2"], inputs["navg_w"], inputs["navg_b"],
        inputs["cavg_w"], inputs["cavg_b"], inputs["out_w"], inputs["out_b"])

    if "nc" not in _CACHE:
        _CACHE["out_b"] = out_bias
        _CACHE["nc"] = _build_program()
    nc = _CACHE["nc"]

    seq_flat = np.zeros((B, FPAD), np.float32)
    seq_flat[:, :FLAT] = seq.reshape(B, FLAT)
    seq_bf = seq_flat.astype(ml_dtypes.bfloat16)
    plen_f = plen.astype(np.float32)

    in_maps = []
    for i in range(N_CORES):
        sh = slice(i * BSH, (i + 1) * BSH)
        m = dict(consts)
        m["seq"] = np.ascontiguousarray(seq_bf[sh])
        m["plen"] = np.ascontiguousarray(plen_f[sh].reshape(NGT, 128).T)
        in_maps.append(m)

    trace = bool(int(os.environ.get("TRN_KERNEL_TRACE", "0")))
    res = run_bass_kernel_spmd(nc, in_maps, list(range(N_CORES)), trace=trace)
    if trace and res.exec_time_ns is not None:
        print(f"HW exec time: {res.exec_time_ns} ns")
        _CACHE["exec_time_ns"] = res.exec_time_ns
        _CACHE["profile"] = res

    out = np.empty((B,), np.float32)
    for i in range(N_CORES):
        arr = np.asarray(res.results[i]["out"], np.float32)   # [128, NGT]
        out[i * BSH:(i + 1) * BSH] = arr.T.reshape(-1)
    return out
